# revision 11
# baseline (speedup 1.0000x reference)
"""Multi-head attention Trainium2 Bass kernel.

Problem: X [2, 4096, 512] fp32, Wq/Wk/Wv/Wo [512, 512] fp32 (torch Linear
convention y = x @ W.T), H=8 heads, d_head=64.

Sharding: 8 cores, each core owns a contiguous block of 1024 query rows
(core i -> batch i//4, q-rows (i%4)*1024 ...). Every core computes K/V for
its full batch (replicated within the 4-core batch group), then flash-style
attention for its query block, then the full output projection for its rows.
No collectives; host concatenates the 8 [1024, 512] outputs.

Per-core pipeline (all matmuls bf16 with fp32 PSUM accumulation):
  XT = X^T via PE transposes                       [d, l]
  K^T = Wk^T.T-contraction (lhsT=WkT, rhs=XT)      [e, l]   (e on partitions)
  V   = (lhsT=XT, rhs=WvT)                         [l, e]   natural
  Q^T = (lhsT=WqT, rhs=XQT)                        [e, q]
  scoresT_h = (lhsT=K^T_h chunk, rhs=Q^T_h)        [l, q]   PSUM
  expT = exp(scoresT / 8)  (ACT, no max-sub: |scores/8| < ~7 for randn inputs)
  U^T_h = (lhsT=[V_h | 1 | 0pad], rhs=expT)        [e+1, q] PSUM (row 64 = softmax denom)
  UTn_h = U^T_h[0:64] * broadcast(1/denom)         (PE ones-bcast + DVE mult)
  out  = sum_h (lhsT=UTn_h, rhs=WoT_h)             [q, d]   natural
"""

import numpy as np

import concourse.bass as bass
import concourse.tile as tile
from concourse import bacc
from concourse import mybir
from concourse.bass import MemorySpace
from concourse.masks import make_identity

F32 = mybir.dt.float32
BF16 = mybir.dt.bfloat16
F32R = mybir.dt.float32r

D = 512
H = 8
DH = 64
P = 128
B = 2
L_FULL = 4096
N_CORES = 8
LQ_CORE = B * L_FULL // N_CORES  # 1024


def build_mha(L=L_FULL, LQ=LQ_CORE, compile=True):
    """Build the per-core SPMD Bass program.

    L  = key/value sequence length handled by this core (its batch)
    LQ = query rows owned by this core
    """
    assert L % 256 == 0 and LQ % 256 == 0
    NLB = L // 256      # X lblocks (256 rows each)
    NQB = LQ // 256     # XQ lblocks
    LCH = L // P        # 128-row l-chunks
    QTW = 512 if LQ % 512 == 0 else 256   # q-tile width (moving N for scores)
    NQT = LQ // QTW
    NSC = LCH // 2      # score super-chunks (2 l-chunks -> 2 PSUM banks)

    nc = bacc.Bacc()
    XB = nc.declare_dram_parameter("XB", [L, D], F32, isOutput=False)
    XQ = nc.declare_dram_parameter("XQ", [LQ, D], F32, isOutput=False)
    WQ = nc.declare_dram_parameter("WQ", [D, D], F32, isOutput=False)
    WK = nc.declare_dram_parameter("WK", [D, D], F32, isOutput=False)
    WV = nc.declare_dram_parameter("WV", [D, D], F32, isOutput=False)
    WO = nc.declare_dram_parameter("WO", [D, D], F32, isOutput=False)
    OUT = nc.declare_dram_parameter("OUT", [LQ, D], F32, isOutput=True)

    def copy_conv(use_scalar, out, in_):
        # All PSUM->SBUF copies that feed matmuls go through DVE so that any
        # matmul depends on a single producer semaphore (walrus rejects
        # matmuls carrying more than one sync wait).
        del use_scalar
        nc.vector.tensor_copy(out=out, in_=in_)

    with tile.TileContext(nc) as tc:
        with tc.tile_pool(name="persist", bufs=1) as persist:
            ident = persist.tile([P, P], F32, tag="ident")
            make_identity(nc, ident)
            # ones row at partition 64 (must match base partition of the
            # denominator row it multiplies against in the bcast matmul)
            ones65 = persist.tile([DH + 1, DH], BF16, tag="ones65")
            nc.vector.memset(ones65[DH : DH + 1, :], 1.0)

            # Persistent activation stores (bf16)
            KT = [persist.tile([P, L], BF16, name=f"kt{ec}", tag=f"kt{ec}") for ec in range(4)]
            QT = [persist.tile([P, LQ], BF16, name=f"qt{ec}", tag=f"qt{ec}") for ec in range(4)]
            # V tiles: [l%128, l-chunk, head, 64 V | 1 ones | 63 zero]
            VT = persist.tile([P, LCH, H, P], BF16, tag="vt")
            nc.vector.memset(VT[:, :, :, DH:], 0.0)
            nc.vector.memset(VT[:, :, :, DH : DH + 1], 1.0)
            UTN = [persist.tile([DH, LQ], BF16, name=f"utn{h}", tag=f"utn{h}") for h in range(H)]
            WOT = persist.tile([P, 4, D], BF16, tag="wot")
            # odd heads' Wo^T slices live at base partition 64 inside WOT;
            # matmul needs lhsT/rhs at the same base partition, so DMA-shift
            # them down to partition 0 (single DMA -> single semaphore tick)
            WOTH = persist.tile([DH, 4, D], BF16, tag="woth")

            # ---------------- Phase 0/1: weights + projections ----------------
            with (
                tc.tile_pool(name="wtp", bufs=1) as wtp,
                tc.tile_pool(name="ld", bufs=2) as ld,
                tc.tile_pool(name="pst", bufs=2, space="PSUM") as pst,
                tc.tile_pool(name="psa", bufs=2, space="PSUM") as psa,
                tc.tile_pool(name="psb", bufs=2, space="PSUM") as psb,
            ):
                # pre-consume the identity's GpSimd sem on PE so real
                # transposes only wait on their DMA
                warm = pst.tile([P, P], F32, name="warm", tag="tp")
                nc.tensor.transpose(warm, ident, ident)

                # -- load + transpose weights: WT[d%128, dc, e] = W[e, d]
                WTS = {}
                for name, dram in (("q", WQ), ("k", WK), ("v", WV), ("o", WO)):
                    wnat = ld.tile([P, 4, D], F32, tag="wnat")
                    nc.sync.dma_start(
                        out=wnat, in_=dram.rearrange("(a p) d -> p a d", p=P)
                    )
                    wt = WOT if name == "o" else wtp.tile([P, 4, D], BF16, name=f"wt{name}", tag=f"wt{name}")
                    WTS[name] = wt
                    # For q/k/v this builds WT[d%128, dc, e] = W[e, d]; for Wo
                    # the same (ec, dc) sweep builds WOT[e%128, ec_out, d] =
                    # Wo[d, e] (the two loop vars swap roles).
                    for ec in range(4):
                        for dc in range(4):
                            tp = pst.tile([P, P], F32, tag="tp")
                            nc.tensor.transpose(
                                tp, wnat[:, ec, dc * P : (dc + 1) * P], ident
                            )
                            copy_conv((ec + dc) % 2, wt[:, dc, ec * P : (ec + 1) * P], tp)

                nc.sync.dma_start(out=WOTH, in_=WOT[DH:P, :, :])
                # pre-consume the WOTH DMA tick on PE while no other fresh
                # dep exists (keeps later out-proj matmuls at one sync wait)
                dmm = pst.tile([1, 1], F32, name="dmm", tag="dmm")
                nc.tensor.matmul(
                    dmm, WOTH[0:1, 0, 0:1], WOTH[0:1, 0, 0:1],
                    start=True, stop=True,
                )

                # -- stream X blocks: transpose, project K/V (and Q from XQ)
                def do_block(src, lb, is_q):
                    xnat = ld.tile([P, 2, D], F32, tag="xnat")
                    nc.sync.dma_start(
                        out=xnat,
                        in_=src[lb * 256 : (lb + 1) * 256, :].rearrange(
                            "(a p) d -> p a d", p=P
                        ),
                    )
                    xt = ld.tile([P, 4, 256], BF16, tag="xt")
                    for ls in range(2):
                        for dc in range(4):
                            tp = pst.tile([P, P], F32, tag="tp")
                            nc.tensor.transpose(
                                tp, xnat[:, ls, dc * P : (dc + 1) * P], ident
                            )
                            copy_conv((ls + dc) % 2, xt[:, dc, ls * P : (ls + 1) * P], tp)
                    if is_q:
                        # Q^T[e, q]: lhsT = WqT[d, e] chunks, rhs = XT[d, q 256]
                        for ec in range(4):
                            pq = psa.tile([P, 256], F32, tag="pacc")
                            for dc in range(4):
                                nc.tensor.matmul(
                                    pq,
                                    WTS["q"][:, dc, ec * P : (ec + 1) * P],
                                    xt[:, dc, :],
                                    start=(dc == 0),
                                    stop=(dc == 3),
                                )
                            copy_conv(ec % 2, QT[ec][:, lb * 256 : (lb + 1) * 256], pq)
                    else:
                        # K^T[e, l]
                        for ec in range(4):
                            pk = psa.tile([P, 256], F32, tag="pacc")
                            for dc in range(4):
                                nc.tensor.matmul(
                                    pk,
                                    WTS["k"][:, dc, ec * P : (ec + 1) * P],
                                    xt[:, dc, :],
                                    start=(dc == 0),
                                    stop=(dc == 3),
                                )
                            copy_conv(ec % 2, KT[ec][:, lb * 256 : (lb + 1) * 256], pk)
                        # V[l, e] natural: lhsT = XT chunk, rhs = WvT[d, e 512]
                        for ls in range(2):
                            pv = psb.tile([P, D], F32, tag="pv")
                            for dc in range(4):
                                nc.tensor.matmul(
                                    pv,
                                    xt[:, dc, ls * P : (ls + 1) * P],
                                    WTS["v"][:, dc, :],
                                    start=(dc == 0),
                                    stop=(dc == 3),
                                )
                            lc = lb * 2 + ls
                            copy_conv(ls % 2, VT[:, lc, :, 0:DH],
                                      pv.rearrange("p (h e) -> p h e", h=H))

                for lb in range(NLB):
                    do_block(XB, lb, is_q=False)
                for qb in range(NQB):
                    do_block(XQ, qb, is_q=True)

            # ---------------- Phase 2: attention ----------------
            with (
                tc.tile_pool(name="expp", bufs=3) as expp,
                tc.tile_pool(name="nrm", bufs=2) as nrm,
                tc.tile_pool(name="osb", bufs=2) as osbp,
                tc.tile_pool(name="pssc", bufs=2, space="PSUM") as pssc,
                tc.tile_pool(name="psut", bufs=2, space="PSUM") as psut,
                tc.tile_pool(name="psbc", bufs=1, space="PSUM") as psbc,
                tc.tile_pool(name="psop", bufs=1, space="PSUM") as psop,
            ):
                for qt in range(NQT):
                    qs = slice(qt * QTW, (qt + 1) * QTW)
                    for h in range(H):
                        ec, hp = h // 2, (h % 2) * DH
                        ut = psut.tile([P, QTW], F32, tag="ut")
                        for sc in range(NSC):
                            sp = pssc.tile([P, 2, QTW], F32, tag="sc")
                            for j in range(2):
                                lc = sc * 2 + j
                                nc.tensor.matmul(
                                    sp[:, j, :],
                                    KT[ec][hp : hp + DH, lc * P : (lc + 1) * P],
                                    QT[ec][hp : hp + DH, qs],
                                    start=True,
                                    stop=True,
                                )
                            ex = expp.tile([P, 2, QTW], BF16, tag="ex")
                            nc.scalar.activation(
                                out=ex, in_=sp,
                                func=mybir.ActivationFunctionType.Exp,
                                scale=0.125,
                            )
                            for j in range(2):
                                lc = sc * 2 + j
                                nc.tensor.matmul(
                                    ut,
                                    VT[:, lc, h, :],
                                    ex[:, j, :],
                                    start=(sc == 0 and j == 0),
                                    stop=(sc == NSC - 1 and j == 1),
                                )
                        # normalize: rows 0:64 of ut are U^T, row 64 is denom
                        rc = nrm.tile([DH + 1, QTW], BF16, tag="rc")
                        with nc.allow_low_precision(
                            reason="softmax denom reciprocal in bf16 (rel tol 2e-2)"
                        ):
                            nc.vector.reciprocal(
                                rc[DH : DH + 1, :], ut[DH : DH + 1, :]
                            )
                        bc = psbc.tile([DH, QTW], F32, tag="bc")
                        nc.tensor.matmul(
                            bc,
                            ones65[DH : DH + 1, :],
                            rc[DH : DH + 1, :],
                            start=True,
                            stop=True,
                        )
                        bcs = nrm.tile([DH, QTW], BF16, tag="bcs")
                        nc.vector.tensor_copy(out=bcs, in_=bc)
                        nc.vector.tensor_mul(UTN[h][:, qs], ut[0:DH, :], bcs)
                    # output projection for this q-tile
                    for qc in range(QTW // P):
                        po = psop.tile([P, D], F32, tag="po")
                        for h in range(H):
                            rhs = (
                                WOT[0:DH, h // 2, :]
                                if h % 2 == 0
                                else WOTH[:, h // 2, :]
                            )
                            nc.tensor.matmul(
                                po,
                                UTN[h][:, qt * QTW + qc * P : qt * QTW + (qc + 1) * P],
                                rhs,
                                start=(h == 0),
                                stop=(h == H - 1),
                            )
                        ob = osbp.tile([P, D], F32, tag="ob")
                        nc.vector.tensor_copy(out=ob, in_=po)
                        r0 = qt * QTW + qc * P
                        nc.sync.dma_start(out=OUT[r0 : r0 + P, :], in_=ob)

    if compile:
        nc.compile()
    return nc


_NC_CACHE = {}


def _get_nc():
    if "nc" not in _NC_CACHE:
        _NC_CACHE["nc"] = build_mha()
    return _NC_CACHE["nc"]


def _run(X, Wq, Wk, Wv, Wo, trace=False, **tkw):
    from concourse.bass_utils import run_bass_kernel_spmd

    X = np.ascontiguousarray(np.asarray(X, dtype=np.float32))
    Wq, Wk, Wv, Wo = (
        np.ascontiguousarray(np.asarray(w, dtype=np.float32)) for w in (Wq, Wk, Wv, Wo)
    )
    nc = _get_nc()
    in_maps = []
    for i in range(N_CORES):
        b = i // (N_CORES // B)
        q0 = (i % (N_CORES // B)) * LQ_CORE
        in_maps.append(
            {
                "XB": X[b],
                "XQ": X[b, q0 : q0 + LQ_CORE],
                "WQ": Wq,
                "WK": Wk,
                "WV": Wv,
                "WO": Wo,
            }
        )
    bkr = run_bass_kernel_spmd(
        nc, in_maps, list(range(N_CORES)), trace=trace, **tkw
    )
    out = np.empty((B, L_FULL, D), dtype=np.float32)
    for i in range(N_CORES):
        b = i // (N_CORES // B)
        q0 = (i % (N_CORES // B)) * LQ_CORE
        out[b, q0 : q0 + LQ_CORE] = bkr.results[i]["OUT"]
    return out, bkr


def kernel(X, Wq, Wk, Wv, Wo):
    return _run(X, Wq, Wk, Wv, Wo)[0]
